# revision 1
# baseline (speedup 1.0000x reference)
"""ContrastiveSWM forward kernel for 8 trn2 NeuronCores (Bass/Tile).

Data-parallel over batch (512 samples/core). Host does layout prep only
(im2col permutation of obs — patches are disjoint since stride==kernel —
plus weight reshapes and action one-hot). Device does all FLOPs:
  conv1 (as matmul) -> train-mode BN (batch stats via tiny AllReduce)
  -> ReLU -> conv2 1x1 -> sigmoid -> encoder MLP -> edge MLP + segment
  sum (via PSUM accumulation) -> node MLP -> state + delta.
Matmuls run in float32r (hw-rounded fp32, ~1e-4 rel err, full PE rate).
"""
import sys

sys.path.insert(0, "/opt/trn_rl_repo")

import numpy as np
import concourse.bass as bass
from concourse import bacc
import concourse.mybir as mybir
import concourse.tile as tile
from concourse.bass_utils import run_bass_kernel_spmd
from concourse.masks import make_identity

F32 = mybir.dt.float32
F32R = mybir.dt.float32r
AF = mybir.ActivationFunctionType
OP = mybir.AluOpType

NCORES = 8
B, K, EMB, HID, ADIM = 4096, 5, 128, 512, 4
BL = B // NCORES          # 512 samples per core
FEAT = 25                 # 5x5 spatial feature map = encoder input dim
CIN = 300                 # 3*10*10 patch size
CINP = 384                # padded to 3*128
NP_TILES = FEAT           # one col-tile per (i,j): [384, 512] patches
TOK = BL * K              # 2560 node tokens per core
NCH = 20                  # token chunks of 128 (k-major: chunk = (k, bc))
EPS = 1e-5

ROW = [0, 0, 0, 0, 1, 1, 1, 1, 2, 2, 2, 2, 3, 3, 3, 3, 4, 4, 4, 4]
COL = [1, 2, 3, 4, 0, 2, 3, 4, 0, 1, 3, 4, 0, 1, 2, 4, 0, 1, 2, 3]

_PROGRAM_CACHE = {}


def _build_program():
    nc = bacc.Bacc()

    # ---------------- dram I/O (per core) ----------------
    xT_d = nc.dram_tensor("xT", [CINP, FEAT, BL], F32R, kind="ExternalInput")
    w1c_d = nc.dram_tensor("w1c", [128, 3, HID], F32R, kind="ExternalInput")
    bng_d = nc.dram_tensor("bng", [128, 4], F32, kind="ExternalInput")
    bnb_d = nc.dram_tensor("bnb", [128, 4], F32, kind="ExternalInput")
    w2c_d = nc.dram_tensor("w2c", [128, 4, K], F32R, kind="ExternalInput")
    b2c_d = nc.dram_tensor("b2c", [K, 1], F32, kind="ExternalInput")
    aohT_d = nc.dram_tensor("aohT", [ADIM, K, BL], F32R, kind="ExternalInput")

    ew = {}
    for name, shape in [
        ("enc_w1", [FEAT, HID]), ("enc_w2", [128, 4, HID]),
        ("enc_w3", [128, 4, EMB]),
        ("edge_w1t", [128, HID]), ("edge_w1b", [128, HID]),
        ("edge_w2", [128, 4, HID]), ("edge_w3", [128, 4, HID]),
        ("node_w1s", [128, HID]), ("node_w1a", [ADIM, HID]),
        ("node_w1g", [128, 4, HID]), ("node_w2", [128, 4, HID]),
        ("node_w3", [128, 4, EMB]),
    ]:
        ew[name] = nc.dram_tensor(name, shape, F32R, kind="ExternalInput")
    rows = {}
    for name, n in [
        ("enc_b1", HID), ("enc_b2", HID), ("enc_b3", EMB),
        ("edge_b1", HID), ("edge_b2", HID), ("edge_b3x4", HID),
        ("node_b1", HID), ("node_b2", HID), ("node_b3", EMB),
        ("enc_g", HID), ("enc_be", HID), ("edge_g", HID), ("edge_be", HID),
        ("node_g", HID), ("node_be", HID),
    ]:
        rows[name] = nc.dram_tensor(name, [n], F32, kind="ExternalInput")

    out_d = nc.dram_tensor("out", [BL, K, EMB], F32, kind="ExternalOutput")

    s_d = nc.dram_tensor("s_bounce", [K, FEAT, BL], F32)  # sigmoid feats
    cc_in = nc.dram_tensor("cc_in", [128, 8], F32)
    cc_out = nc.dram_tensor("cc_out", [128, 8], F32, addr_space="Shared")

    with tile.TileContext(nc) as tc:
        _emit(nc, tc, xT_d, w1c_d, bng_d, bnb_d, w2c_d, b2c_d, aohT_d,
              ew, rows, out_d, s_d, cc_in, cc_out)
    nc.finalize()
    return nc


def _emit(nc, tc, xT_d, w1c_d, bng_d, bnb_d, w2c_d, b2c_d, aohT_d,
          ew, rows, out_d, s_d, cc_in, cc_out):
    from contextlib import ExitStack

    ctx = ExitStack()
    with ctx:
        const = ctx.enter_context(tc.tile_pool(name="const", bufs=1))
        sm = ctx.enter_context(tc.tile_pool(name="small", bufs=1))

        # identity for PE transposes (f32r via DVE cast)
        ident_f = const.tile([128, 128], F32)
        make_identity(nc, ident_f[:])
        ident = const.tile([128, 128], F32R)
        nc.vector.tensor_copy(out=ident[:], in_=ident_f[:])

        eps_t = const.tile([128, 1], F32)
        nc.vector.memset(eps_t[:], EPS)

        def bc_row(src_h, n, _tag=[0]):
            _tag[0] += 1
            dst = const.tile([128, n], F32, tag=f"bcrow{_tag[0]}")
            ap = src_h.ap()
            bcast = bass.AP(tensor=ap.tensor, offset=ap.offset,
                            ap=[[0, 128]] + ap.ap)
            nc.gpsimd.dma_start(out=dst[:], in_=bcast)
            return dst

        bc = {k: bc_row(v, v.shape[0]) for k, v in rows.items()
              if k != "enc_b3"}
        # enc_b3 is per-partition (emb) in stateT layout -> [128, 1]
        encb3 = const.tile([128, 1], F32)
        nc.sync.dma_start(out=encb3[:], in_=rows["enc_b3"].ap().rearrange("(p one) -> p one", one=1))

        # conv weights + bn params
        w1c = const.tile([128, 3, HID], F32R)
        nc.sync.dma_start(out=w1c[:], in_=w1c_d[:, :, :])
        w2c = const.tile([128, 4, K], F32R)
        nc.sync.dma_start(out=w2c[:], in_=w2c_d[:, :, :])
        b2c = const.tile([K, 1], F32)
        nc.sync.dma_start(out=b2c[:], in_=b2c_d[:, :])
        bng = const.tile([128, 4], F32)
        nc.sync.dma_start(out=bng[:], in_=bng_d[:, :])
        bnb = const.tile([128, 4], F32)
        nc.sync.dma_start(out=bnb[:], in_=bnb_d[:, :])

        # MLP weights
        W = {}
        for name, h in ew.items():
            t = const.tile(list(h.shape), F32R, tag=f"w_{name}")
            nc.sync.dma_start(out=t[:], in_=h[tuple([slice(None)] * len(h.shape))])
            W[name] = t
        aohT = const.tile([ADIM, K, BL], F32R)
        nc.sync.dma_start(out=aohT[:], in_=aohT_d[:, :, :])

        xT_v = xT_d.rearrange("(kc p) ij b -> p kc ij b", p=128)

        # ================= conv phase =================
        stats_all = sm.tile([128, 4, FEAT, 6], F32)
        P = sm.tile([128, 4, 2], F32)

        with (
            tc.tile_pool(name="xtile", bufs=3) as xpool,
            tc.tile_pool(name="hps", bufs=3, space="PSUM") as hps,
        ):
            # ---- pass 1: conv1 (no bias) -> per-channel batch stats ----
            for ij in range(NP_TILES):
                xt = xpool.tile([128, 3, BL], F32R)
                nc.sync.dma_start(out=xt[:], in_=xT_v[:, :, ij, :])
                for cc in range(4):
                    hp = hps.tile([128, BL], F32)
                    for kc in range(3):
                        nc.tensor.matmul(
                            hp[:], w1c[:, kc, bass.ts(cc, 128)], xt[:, kc, :],
                            start=(kc == 0), stop=(kc == 2),
                        )
                    nc.vector.bn_stats(out=stats_all[:, cc, ij, :], in_=hp[:])

        # aggregate per-channel mean/var; q = var + mean^2
        for cc in range(4):
            nc.vector.bn_aggr(out=P[:, cc, :], in_=stats_all[:, cc, :, :])
        msq = sm.tile([128, 4], F32)
        nc.vector.tensor_mul(msq[:], P[:, :, 0], P[:, :, 0])
        nc.vector.tensor_add(P[:, :, 1], P[:, :, 1], msq[:])

        nc.sync.dma_start(out=cc_in[:, :], in_=P[:].rearrange("p a b -> p (a b)"))
        nc.gpsimd.collective_compute(
            "AllReduce", OP.add,
            replica_groups=[list(range(NCORES))],
            ins=[cc_in[:, :]], outs=[cc_out[:, :]],
        )
        G = sm.tile([128, 4, 2], F32)
        nc.sync.dma_start(out=G[:].rearrange("p a b -> p (a b)"), in_=cc_out[:, :])

        mg = sm.tile([128, 4], F32)
        qg = sm.tile([128, 4], F32)
        nc.vector.tensor_scalar(out=mg[:], in0=G[:, :, 0], scalar1=1.0 / NCORES,
                                scalar2=None, op0=OP.mult)
        nc.vector.tensor_scalar(out=qg[:], in0=G[:, :, 1], scalar1=1.0 / NCORES,
                                scalar2=None, op0=OP.mult)
        varg = sm.tile([128, 4], F32)
        nc.vector.tensor_mul(varg[:], mg[:], mg[:])
        nc.vector.tensor_sub(varg[:], qg[:], varg[:])
        sd = sm.tile([128, 4], F32)
        nc.scalar.activation(sd[:], varg[:], AF.Sqrt, bias=eps_t[:, 0:1])
        rstd = sm.tile([128, 4], F32)
        nc.vector.reciprocal(rstd[:], sd[:])
        scale = sm.tile([128, 4], F32)
        nc.vector.tensor_mul(scale[:], bng[:], rstd[:])
        shift = sm.tile([128, 4], F32)
        nc.vector.tensor_mul(shift[:], mg[:], scale[:])
        nc.vector.tensor_sub(shift[:], bnb[:], shift[:])

        # ---- pass 2: conv1 -> BN+ReLU -> conv2 -> sigmoid -> s_d ----
        with (
            tc.tile_pool(name="xtile2", bufs=3) as xpool,
            tc.tile_pool(name="hps2", bufs=3, space="PSUM") as hps,
            tc.tile_pool(name="hbn", bufs=2) as hbnp,
            tc.tile_pool(name="sps", bufs=2, space="PSUM") as sps,
            tc.tile_pool(name="ssb", bufs=3) as ssb,
        ):
            for ij in range(NP_TILES):
                xt = xpool.tile([128, 3, BL], F32R)
                nc.sync.dma_start(out=xt[:], in_=xT_v[:, :, ij, :])
                hbn = hbnp.tile([128, 4, BL], F32R)
                for cc in range(4):
                    hp = hps.tile([128, BL], F32)
                    for kc in range(3):
                        nc.tensor.matmul(
                            hp[:], w1c[:, kc, bass.ts(cc, 128)], xt[:, kc, :],
                            start=(kc == 0), stop=(kc == 2),
                        )
                    # BN + ReLU: relu(h*scale + shift), per-partition consts
                    nc.vector.tensor_scalar(
                        out=hbn[:, cc, :], in0=hp[:],
                        scalar1=scale[:, cc:cc + 1], scalar2=shift[:, cc:cc + 1],
                        op0=OP.mult, op1=OP.add,
                    )
                    nc.scalar.activation(hbn[:, cc, :], hbn[:, cc, :], AF.Relu)
                sp = sps.tile([K, BL], F32)
                for cc in range(4):
                    nc.tensor.matmul(
                        sp[:], w2c[:, cc, :], hbn[:, cc, :],
                        start=(cc == 0), stop=(cc == 3),
                    )
                s_sb = ssb.tile([K, BL], F32)
                nc.scalar.activation(s_sb[:], sp[:], AF.Sigmoid,
                                     bias=b2c[:, 0:1])
                nc.sync.dma_start(out=s_d[:, ij, :], in_=s_sb[:])

        # ============ encoder feats reload (transposed via DRAM) ============
        xenc = const.tile([FEAT, K, BL], F32R)
        nc.gpsimd.dma_start(
            out=xenc[:],
            in_=s_d.rearrange("k ij b -> ij k b"),
        )

        out_v = out_d.rearrange("(c p) k e -> p c k e", p=128)
        stateT = const.tile([128, K, BL], F32R)   # [emb, (k,b)]
        state_tm = const.tile([128, NCH, EMB], F32)  # [b%128, chunk, emb]

        mm1 = ctx.enter_context(tc.tile_pool(name="mm1", bufs=2, space="PSUM"))
        mmT = ctx.enter_context(tc.tile_pool(name="mmT", bufs=3, space="PSUM"))
        mm2 = ctx.enter_context(tc.tile_pool(name="mm2", bufs=2, space="PSUM"))
        agg = ctx.enter_context(tc.tile_pool(name="agg", bufs=1, space="PSUM"))
        work = ctx.enter_context(tc.tile_pool(name="work", bufs=2))
        chain = ctx.enter_context(tc.tile_pool(name="chain", bufs=3))
        s2w = ctx.enter_context(tc.tile_pool(name="s2w", bufs=1))
        uvp = ctx.enter_context(tc.tile_pool(name="uvp", bufs=1))
        aggs = ctx.enter_context(tc.tile_pool(name="aggs", bufs=1))

        def layer_norm_relu(p2, b2bc, gbc, bebc, out_dt=F32R):
            """psum [128,512] -> relu(LN(psum + b2)*g + be) -> sbuf tile."""
            h2 = chain.tile([128, HID], F32, tag="ln_h2")
            nc.vector.tensor_tensor(out=h2[:], in0=p2[:], in1=b2bc[:],
                                    op=OP.add)
            st6 = work.tile([128, 6], F32, tag="ln_st")
            nc.vector.bn_stats(out=st6[:], in_=h2[:])
            mv = work.tile([128, 2], F32, tag="ln_mv")
            nc.vector.bn_aggr(out=mv[:], in_=st6[:])
            sdv = work.tile([128, 1], F32, tag="ln_sd")
            nc.scalar.activation(sdv[:], mv[:, 1:2], AF.Sqrt,
                                 bias=eps_t[:, 0:1])
            rs = work.tile([128, 1], F32, tag="ln_rs")
            nc.vector.reciprocal(rs[:], sdv[:])
            mr = work.tile([128, 1], F32, tag="ln_mr")
            nc.vector.tensor_mul(mr[:], mv[:, 0:1], rs[:])
            xn = chain.tile([128, HID], F32, tag="ln_xn")
            nc.vector.tensor_scalar(out=xn[:], in0=h2[:], scalar1=rs[:],
                                    scalar2=mr[:], op0=OP.mult,
                                    op1=OP.subtract)
            nc.vector.tensor_mul(xn[:], xn[:], gbc[:])
            nc.vector.tensor_add(xn[:], xn[:], bebc[:])
            e2 = chain.tile([128, HID], out_dt, tag="ln_out")
            nc.scalar.activation(e2[:], xn[:], AF.Relu)
            return e2

        def transpose_512(src, dst_dt=F32R, tag="tT"):
            """sbuf [128,512] f32r -> sbuf [128,4,128] transposed chunks."""
            pt = mmT.tile([128, 4, 128], F32R, tag="tps")
            for c in range(4):
                nc.tensor.transpose(pt[:, c, :], src[:, bass.ts(c, 128)],
                                    ident[:])
            dT = chain.tile([128, 4, 128], dst_dt, tag="tsb")
            nc.scalar.activation(dT[:], pt[:], AF.Copy)
            return dT

        # ================= encoder =================
        for g in range(K):  # token group g == object k, 512 tokens
            s2T = s2w.tile([128, 4, BL], F32R, tag="s2T")
            for cc in range(4):  # token chunk within group
                p1 = mm1.tile([128, HID], F32, tag="mm1")
                nc.tensor.matmul(p1[:], xenc[:, g, bass.ts(cc, 128)],
                                 W["enc_w1"][:, :], start=True, stop=True)
                e1 = chain.tile([128, HID], F32R, tag="e1")
                nc.vector.tensor_tensor(out=e1[:], in0=p1[:],
                                        in1=bc["enc_b1"][:], op=OP.add)
                nc.scalar.activation(e1[:], e1[:], AF.Relu)
                e1T = transpose_512(e1, tag="encT1")
                p2 = mm2.tile([128, HID], F32, tag="mm2")
                for kc in range(4):
                    nc.tensor.matmul(p2[:], e1T[:, kc, :],
                                     W["enc_w2"][:, kc, :],
                                     start=(kc == 0), stop=(kc == 3))
                e2 = layer_norm_relu(p2, bc["enc_b2"], bc["enc_g"],
                                     bc["enc_be"])
                # T2 into the wide buffer (columns cc*128..)
                pt = mmT.tile([128, 4, 128], F32R, tag="tps")
                for c in range(4):
                    nc.tensor.transpose(pt[:, c, :], e2[:, bass.ts(c, 128)],
                                        ident[:])
                nc.scalar.activation(s2T[:, :, bass.ts(cc, 128)], pt[:],
                                     AF.Copy)
            # L3 (feature-major wide): stateT[:, g, :] = w3.T @ s2T + b3
            p3 = agg.tile([128, BL], F32, tag="agg")
            for kc in range(4):
                nc.tensor.matmul(p3[:], W["enc_w3"][:, kc, :], s2T[:, kc, :],
                                 start=(kc == 0), stop=(kc == 3))
            nc.vector.tensor_scalar(out=stateT[:, g, :], in0=p3[:],
                                    scalar1=encb3[:, 0:1], scalar2=None,
                                    op0=OP.add)
            # token-major state for final add + output
            for cc in range(4):
                pt = mmT.tile([128, 4, 128], F32R, tag="tps")
                nc.tensor.transpose(pt[:, 0, :],
                                    stateT[:, g, bass.ts(cc, 128)], ident[:])
                nc.scalar.activation(state_tm[:, g * 4 + cc, :],
                                     pt[:, 0, :].bitcast(F32), AF.Copy)

        # ================= edge MLP + aggregation + node MLP =================
        for bc_i in range(4):
            bsl = bass.ts(bc_i, 128)
            # U'_k = state_k @ W1t + b1 ; V_k = state_k @ W1b
            UV = uvp.tile([128, 2, K, HID], F32R, tag="UV")
            for k in range(K):
                pu = mm1.tile([128, HID], F32, tag="mm1")
                nc.tensor.matmul(pu[:], stateT[:, k, bsl],
                                 W["edge_w1t"][:, :], start=True, stop=True)
                nc.vector.tensor_tensor(out=UV[:, 0, k, :], in0=pu[:],
                                        in1=bc["edge_b1"][:], op=OP.add)
                pv = mm1.tile([128, HID], F32, tag="mm1")
                nc.tensor.matmul(pv[:], stateT[:, k, bsl],
                                 W["edge_w1b"][:, :], start=True, stop=True)
                nc.scalar.activation(UV[:, 1, k, :], pv[:], AF.Copy)

            agg_sb = aggs.tile([128, K, HID], F32R, tag="aggsb")
            for r in range(K):
                pagg = agg.tile([128, HID], F32, tag="agg")
                for ei in range(4):
                    e = 4 * r + ei
                    c = COL[e]
                    e1 = chain.tile([128, HID], F32R, tag="e1")
                    nc.vector.tensor_tensor(out=e1[:], in0=UV[:, 0, r, :],
                                            in1=UV[:, 1, c, :], op=OP.add)
                    nc.scalar.activation(e1[:], e1[:], AF.Relu)
                    e1T = transpose_512(e1, tag="edgeT1")
                    p2 = mm2.tile([128, HID], F32, tag="mm2")
                    for kc in range(4):
                        nc.tensor.matmul(p2[:], e1T[:, kc, :],
                                         W["edge_w2"][:, kc, :],
                                         start=(kc == 0), stop=(kc == 3))
                    e2 = layer_norm_relu(p2, bc["edge_b2"], bc["edge_g"],
                                         bc["edge_be"])
                    e2T = transpose_512(e2, tag="edgeT2")
                    for kc in range(4):
                        nc.tensor.matmul(pagg[:], e2T[:, kc, :],
                                         W["edge_w3"][:, kc, :],
                                         start=(ei == 0 and kc == 0),
                                         stop=(ei == 3 and kc == 3))
                # agg(+4*b3) for node r, batch-chunk bc_i
                nc.vector.tensor_tensor(out=agg_sb[:, r, :], in0=pagg[:],
                                        in1=bc["edge_b3x4"][:], op=OP.add)

            # ---- node MLP for tokens (k, bc_i) ----
            for k in range(K):
                aggT = transpose_512(agg_sb[:, k, :], tag="nodeTa")
                p1 = mm1.tile([128, HID], F32, tag="mm1")
                nc.tensor.matmul(p1[:], stateT[:, k, bsl],
                                 W["node_w1s"][:, :], start=True, stop=False)
                nc.tensor.matmul(p1[:], aohT[:, k, bsl],
                                 W["node_w1a"][:, :], start=False, stop=False)
                for kc in range(4):
                    nc.tensor.matmul(p1[:], aggT[:, kc, :],
                                     W["node_w1g"][:, kc, :],
                                     start=False, stop=(kc == 3))
                e1 = chain.tile([128, HID], F32R, tag="e1")
                nc.vector.tensor_tensor(out=e1[:], in0=p1[:],
                                        in1=bc["node_b1"][:], op=OP.add)
                nc.scalar.activation(e1[:], e1[:], AF.Relu)
                e1T = transpose_512(e1, tag="nodeT1")
                p2 = mm2.tile([128, HID], F32, tag="mm2")
                for kc in range(4):
                    nc.tensor.matmul(p2[:], e1T[:, kc, :],
                                     W["node_w2"][:, kc, :],
                                     start=(kc == 0), stop=(kc == 3))
                e2 = layer_norm_relu(p2, bc["node_b2"], bc["node_g"],
                                     bc["node_be"])
                e2T = transpose_512(e2, tag="nodeT2")
                pd = mm1.tile([128, EMB], F32, tag="mm1")
                for kc in range(4):
                    nc.tensor.matmul(pd[:], e2T[:, kc, :],
                                     W["node_w3"][:, kc, :],
                                     start=(kc == 0), stop=(kc == 3))
                # out = state + delta + b3
                ch = k * 4 + bc_i
                oc = work.tile([128, EMB], F32, tag="outc")
                nc.vector.tensor_tensor(out=oc[:], in0=pd[:],
                                        in1=state_tm[:, ch, :], op=OP.add)
                nc.vector.tensor_add(oc[:], oc[:], bc["node_b3"][:])
                nc.sync.dma_start(
                    out=out_v[:, bc_i, k, :], in_=oc[:])



def _prep_inputs(obs, action, weights):
    """Host-side layout prep (pure permutations/reshapes, no math)."""
    obs = np.ascontiguousarray(obs, dtype=np.float32)
    # im2col-transpose: patches are disjoint (stride 10 == kernel 10)
    a = obs.reshape(NCORES, BL, 3, 5, 10, 5, 10)
    a = a.transpose(0, 2, 4, 6, 3, 5, 1)  # [core, c, di, dj, i, j, b]
    a = np.ascontiguousarray(a).reshape(NCORES, CIN, FEAT, BL)
    xT = np.zeros((NCORES, CINP, FEAT, BL), np.float32)
    xT[:, :CIN] = a

    w = {k: np.asarray(v, dtype=np.float32) for k, v in weights.items()}

    w1c = w["conv1_w"].reshape(HID, CIN).T  # [300, 512]
    w1cp = np.zeros((CINP, HID), np.float32)
    w1cp[:CIN] = w1c
    w1c3 = np.ascontiguousarray(
        w1cp.reshape(3, 128, HID).transpose(1, 0, 2))  # [128,3,512]

    bng = np.ascontiguousarray(w["bn1_g"].reshape(4, 128).T)  # [128,4]
    bnb = np.ascontiguousarray(w["bn1_b"].reshape(4, 128).T)

    w2c = np.ascontiguousarray(
        w["conv2_w"].reshape(K, HID).T.reshape(4, 128, K).transpose(1, 0, 2))
    b2c = w["conv2_b"].reshape(K, 1)

    def kc_tiles(m, kchunks):  # [K_in, N] -> [128, kchunks, N]
        return np.ascontiguousarray(
            m.reshape(kchunks, 128, m.shape[1]).transpose(1, 0, 2))

    act = np.asarray(action).astype(np.int64).reshape(NCORES, BL)
    k_idx = act // ADIM
    a_idx = act % ADIM
    aohT = np.zeros((NCORES, ADIM, K, BL), np.float32)
    core_i = np.repeat(np.arange(NCORES), BL)
    b_i = np.tile(np.arange(BL), NCORES)
    aohT[core_i, a_idx.ravel(), k_idx.ravel(), b_i] = 1.0

    shared = {
        "w1c": w1c3, "bng": bng, "bnb": bnb, "w2c": w2c, "b2c": b2c,
        "enc_w1": w["enc_w1"],
        "enc_w2": kc_tiles(w["enc_w2"], 4),
        "enc_w3": kc_tiles(w["enc_w3"], 4),
        "edge_w1t": w["edge_w1"][:128],
        "edge_w1b": w["edge_w1"][128:],
        "edge_w2": kc_tiles(w["edge_w2"], 4),
        "edge_w3": kc_tiles(w["edge_w3"], 4),
        "node_w1s": w["node_w1"][:EMB],
        "node_w1a": w["node_w1"][EMB:EMB + ADIM],
        "node_w1g": kc_tiles(w["node_w1"][EMB + ADIM:], 4),
        "node_w2": kc_tiles(w["node_w2"], 4),
        "node_w3": kc_tiles(w["node_w3"], 4),
        "enc_b1": w["enc_b1"], "enc_b2": w["enc_b2"], "enc_b3": w["enc_b3"],
        "edge_b1": w["edge_b1"], "edge_b2": w["edge_b2"],
        "edge_b3x4": 4.0 * w["edge_b3"],
        "node_b1": w["node_b1"], "node_b2": w["node_b2"],
        "node_b3": w["node_b3"],
        "enc_g": w["enc_lng"], "enc_be": w["enc_lnb"],
        "edge_g": w["edge_lng"], "edge_be": w["edge_lnb"],
        "node_g": w["node_lng"], "node_be": w["node_lnb"],
    }
    shared = {k: np.ascontiguousarray(v, dtype=np.float32)
              for k, v in shared.items()}

    in_maps = []
    for m in range(NCORES):
        d = dict(shared)
        d["xT"] = xT[m]
        d["aohT"] = np.ascontiguousarray(aohT[m])
        in_maps.append(d)
    return in_maps


def kernel(**inputs):
    obs = inputs["obs"]
    action = inputs["action"]
    weights = {k: v for k, v in inputs.items()
               if k not in ("obs", "action")}
    in_maps = _prep_inputs(obs, action, weights)

    if "nc" not in _PROGRAM_CACHE:
        _PROGRAM_CACHE["nc"] = _build_program()
    nc = _PROGRAM_CACHE["nc"]

    res = run_bass_kernel_spmd(nc, in_maps, list(range(NCORES)))
    out = np.concatenate([res.results[m]["out"] for m in range(NCORES)],
                         axis=0)
    return out.astype(np.float32)


if __name__ == "__main__":
    rng = np.random.default_rng(0)
    fake = {
        "obs": rng.standard_normal((B, 3, 50, 50)).astype(np.float32),
        "action": rng.integers(0, ADIM * K, size=(B,)).astype(np.int64),
    }
    print("built program OK" if _build_program() else "fail")



# revision 6
# speedup vs baseline: 2.0995x; 2.0995x over previous
"""ContrastiveSWM forward kernel for 8 trn2 NeuronCores (Bass/Tile), v2.

Data-parallel over batch (512 samples/core). Host does layout prep only
(im2col permutation, weight reshapes/casts, one-hot action, and the
linear fold W35 = edge_w3 @ node_w1g which removes the agg matmul).

Device pipeline (bf16 matmuls, f32 accumulation/LN math):
  conv1 (as matmul) -> stash h1 (bf16, SBUF) + batch stats
  -> tiny AllReduce -> BN affine+relu in place -> conv2 1x1 -> sigmoid
  -> encoder MLP -> edge MLP (feature-major in, token-major LN,
  transpose, W35 accumulate into node-L1 psum) -> node MLP -> state+delta.

Layout trick: L2 layers consume feature-major activations as the matmul
stationary operand, producing token-major outputs for cheap free-dim
LayerNorm; the post-LN transpose back to feature-major carries the relu
in its PSUM-evacuation copy.
"""
import sys

sys.path.insert(0, "/opt/trn_rl_repo")

import numpy as np
import ml_dtypes
import concourse.bass as bass
from concourse import bacc
import concourse.mybir as mybir
import concourse.tile as tile
from concourse.bass_utils import run_bass_kernel_spmd
from concourse.masks import make_identity

F32 = mybir.dt.float32
BF16 = mybir.dt.bfloat16
NPBF = ml_dtypes.bfloat16
AF = mybir.ActivationFunctionType
OP = mybir.AluOpType

NCORES = 8
B, K, EMB, HID, ADIM = 4096, 5, 128, 512, 4
BL = B // NCORES          # 512 samples per core
FEAT = 25                 # 5x5 spatial feature map
CIN = 300                 # 3*10*10 patch size
CINP = 384                # padded to 3*128
EPS = 1e-5

ROW = [0, 0, 0, 0, 1, 1, 1, 1, 2, 2, 2, 2, 3, 3, 3, 3, 4, 4, 4, 4]
COL = [1, 2, 3, 4, 0, 2, 3, 4, 0, 1, 3, 4, 0, 1, 2, 4, 0, 1, 2, 3]

_PROGRAM_CACHE = {}


def _build_program():
    nc = bacc.Bacc()

    # ---------------- dram I/O (per core) ----------------
    xT_d = nc.dram_tensor("xT", [CINP, FEAT, BL], BF16, kind="ExternalInput")
    aoh_d = nc.dram_tensor("aoh", [ADIM, K, BL], BF16, kind="ExternalInput")

    wb = {}   # bf16 weight tensors
    for name, shape in [
        ("w1c", [128, 3, HID]), ("w2c", [128, 4, K]),
        ("enc_w1", [FEAT, HID]), ("enc_w2", [128, 4, HID]),
        ("enc_w3", [128, 4, EMB]),
        ("edge_w1t", [128, HID]), ("edge_w1b", [128, HID]),
        ("edge_w2", [128, 4, HID]), ("w35", [128, 4, HID]),
        ("node_w1s", [128, HID]), ("node_w1a", [ADIM, HID]),
        ("node_w2", [128, 4, HID]), ("node_w3", [128, 4, EMB]),
    ]:
        wb[name] = nc.dram_tensor(name, shape, BF16, kind="ExternalInput")

    fc = {}   # f32 per-partition bias/param columns
    for name, shape in [
        ("bng", [128, 4]), ("bnb", [128, 4]), ("b2c", [K, 1]),
        ("enc_b1c", [128, 4]), ("edge_b1c", [128, 4]), ("node_b1c", [128, 4]),
        ("enc_b3c", [128, 1]), ("node_b3c", [128, 1]),
    ]:
        fc[name] = nc.dram_tensor(name, shape, F32, kind="ExternalInput")

    rows = {}  # bf16 rows [HID] for broadcast tiles / ones-matmul rhs
    for name in ["enc_b2", "edge_b2", "node_b2",
                 "enc_g", "enc_be", "edge_g", "edge_be", "node_g", "node_be"]:
        rows[name] = nc.dram_tensor(name, [HID], BF16, kind="ExternalInput")

    out_d = nc.dram_tensor("out", [K, EMB, BL], F32, kind="ExternalOutput")

    s_d = nc.dram_tensor("s_bounce", [K, FEAT, BL], BF16)  # sigmoid feats
    cc_in = nc.dram_tensor("cc_in", [128, 8], F32)
    cc_out = nc.dram_tensor("cc_out", [128, 8], F32, addr_space="Shared")

    with tile.TileContext(nc) as tc:
        _emit(nc, tc, xT_d, aoh_d, wb, fc, rows, out_d, s_d, cc_in, cc_out)
    nc.finalize()
    return nc


def _emit(nc, tc, xT_d, aoh_d, wb, fc, rows, out_d, s_d, cc_in, cc_out):
    from contextlib import ExitStack

    ctx = ExitStack()
    with ctx:
        const = ctx.enter_context(tc.tile_pool(name="const", bufs=1))
        sm = ctx.enter_context(tc.tile_pool(name="small", bufs=1))

        xT_v = xT_d.rearrange("(kc p) ij b -> p kc ij b", p=128)

        # conv1 weights first (needed by the very first matmuls)
        w1c = const.tile([128, 3, HID], BF16)
        nc.sync.dma_start(out=w1c[:], in_=wb["w1c"][:, :, :])

        # identity for PE transposes (bf16 via DVE cast)
        ident_f = const.tile([128, 128], F32)
        make_identity(nc, ident_f[:])
        ident = const.tile([128, 128], BF16)
        nc.vector.tensor_copy(out=ident[:], in_=ident_f[:])

        ones1 = const.tile([1, 128], BF16)
        nc.vector.memset(ones1[:], 1.0)
        eps_t = const.tile([128, 1], F32)
        nc.vector.memset(eps_t[:], EPS)

        # remaining weights (gpsimd queue so x tiles stream on sync queue)
        W = {}
        for name, h in wb.items():
            if name == "w1c":
                W[name] = w1c
                continue
            t = const.tile(list(h.shape), BF16, tag=f"w_{name}")
            nc.gpsimd.dma_start(
                out=t[:], in_=h[tuple([slice(None)] * len(h.shape))])
            W[name] = t
        FC = {}
        for name, h in fc.items():
            t = const.tile(list(h.shape), F32, tag=f"fc_{name}")
            nc.gpsimd.dma_start(out=t[:], in_=h[:, :])
            FC[name] = t

        def bc_row(src_h, _tag=[0]):
            """[HID] bf16 dram row -> [128, HID] broadcast tile."""
            _tag[0] += 1
            dst = const.tile([128, HID], BF16, tag=f"bcrow{_tag[0]}")
            ap = src_h.ap()
            bcast = bass.AP(tensor=ap.tensor, offset=ap.offset,
                            ap=[[0, 128]] + ap.ap)
            nc.gpsimd.dma_start(out=dst[:], in_=bcast)
            return dst

        def one_row(src_h, _tag=[0]):
            """[HID] bf16 dram row -> [1, HID] tile (ones-matmul rhs)."""
            _tag[0] += 1
            dst = const.tile([1, HID], BF16, tag=f"onerow{_tag[0]}")
            nc.gpsimd.dma_start(
                out=dst[:], in_=src_h.ap().rearrange("(o f) -> o f", o=1))
            return dst

        g_bc = {m: bc_row(rows[f"{m}_g"]) for m in ("enc", "edge", "node")}
        be_bc = {m: bc_row(rows[f"{m}_be"]) for m in ("enc", "edge", "node")}
        b2_row = {m: one_row(rows[f"{m}_b2"]) for m in ("enc", "edge", "node")}

        aohbf = const.tile([ADIM, K, BL], BF16)
        nc.gpsimd.dma_start(out=aohbf[:], in_=aoh_d[:, :, :])

        # ================= conv phase =================
        stats_all = sm.tile([128, 4, FEAT, 6], F32)
        P = sm.tile([128, 4, 2], F32)

        with (
            tc.tile_pool(name="xtile", bufs=3) as xpool,
            tc.tile_pool(name="cps", bufs=3, space="PSUM") as cps,
        ):
            # conv1 (no bias) -> stash + per-channel batch stats
            for ij in range(FEAT):
                xt = xpool.tile([128, 3, BL], BF16)
                nc.sync.dma_start(out=xt[:], in_=xT_v[:, :, ij, :])
                for cc in range(4):
                    hp = cps.tile([128, BL], F32)
                    for kc in range(3):
                        nc.tensor.matmul(
                            hp[:], w1c[:, kc, bass.ts(cc, 128)], xt[:, kc, :],
                            start=(kc == 0), stop=(kc == 2),
                        )
                    nc.vector.bn_stats(out=stats_all[:, cc, ij, :], in_=hp[:])

        # aggregate per-channel mean/var; q = var + mean^2
        for cc in range(4):
            nc.vector.bn_aggr(out=P[:, cc, :], in_=stats_all[:, cc, :, :])
        msq = sm.tile([128, 4], F32)
        nc.vector.tensor_mul(msq[:], P[:, :, 0], P[:, :, 0])
        nc.vector.tensor_add(P[:, :, 1], P[:, :, 1], msq[:])

        nc.sync.dma_start(out=cc_in[:, :], in_=P[:].rearrange("p a b -> p (a b)"))
        nc.gpsimd.collective_compute(
            "AllReduce", OP.add,
            replica_groups=[list(range(NCORES))],
            ins=[cc_in[:, :]], outs=[cc_out[:, :]],
        )
        G = sm.tile([128, 4, 2], F32)
        nc.sync.dma_start(out=G[:].rearrange("p a b -> p (a b)"), in_=cc_out[:, :])

        mg = sm.tile([128, 4], F32)
        qg = sm.tile([128, 4], F32)
        nc.vector.tensor_scalar(out=mg[:], in0=G[:, :, 0], scalar1=1.0 / NCORES,
                                scalar2=None, op0=OP.mult)
        nc.vector.tensor_scalar(out=qg[:], in0=G[:, :, 1], scalar1=1.0 / NCORES,
                                scalar2=None, op0=OP.mult)
        varg = sm.tile([128, 4], F32)
        nc.vector.tensor_mul(varg[:], mg[:], mg[:])
        nc.vector.tensor_sub(varg[:], qg[:], varg[:])
        sd = sm.tile([128, 4], F32)
        nc.scalar.activation(sd[:], varg[:], AF.Sqrt, bias=eps_t[:, 0:1])
        rstd = sm.tile([128, 4], F32)
        nc.vector.reciprocal(rstd[:], sd[:])
        scale = sm.tile([128, 4], F32)
        nc.vector.tensor_mul(scale[:], FC["bng"][:], rstd[:])
        shift = sm.tile([128, 4], F32)
        nc.vector.tensor_mul(shift[:], mg[:], scale[:])
        nc.vector.tensor_sub(shift[:], FC["bnb"][:], shift[:])

        # pass 2: conv1 recompute -> BN+relu (ACT, fused) -> conv2 -> sigmoid
        with (
            tc.tile_pool(name="xtile2", bufs=3) as xpool,
            tc.tile_pool(name="cps2", bufs=3, space="PSUM") as cps,
            tc.tile_pool(name="hbnp", bufs=2) as hbnp,
            tc.tile_pool(name="sps", bufs=2, space="PSUM") as sps,
            tc.tile_pool(name="ssb", bufs=3) as ssb,
        ):
            for ij in range(FEAT):
                xt = xpool.tile([128, 3, BL], BF16)
                nc.sync.dma_start(out=xt[:], in_=xT_v[:, :, ij, :])
                hbn = hbnp.tile([128, 4, BL], BF16, tag="hbn")
                for cc in range(4):
                    hp = cps.tile([128, BL], F32)
                    for kc in range(3):
                        nc.tensor.matmul(
                            hp[:], w1c[:, kc, bass.ts(cc, 128)], xt[:, kc, :],
                            start=(kc == 0), stop=(kc == 2),
                        )
                    nc.scalar.activation(hbn[:, cc, :], hp[:], AF.Relu,
                                         scale=scale[:, cc:cc + 1],
                                         bias=shift[:, cc:cc + 1])
                sp = sps.tile([K, BL], F32)
                for cc in range(4):
                    nc.tensor.matmul(
                        sp[:], W["w2c"][:, cc, :], hbn[:, cc, :],
                        start=(cc == 0), stop=(cc == 3),
                    )
                s_sb = ssb.tile([K, BL], BF16)
                nc.scalar.activation(s_sb[:], sp[:], AF.Sigmoid,
                                     bias=FC["b2c"][:, 0:1])
                nc.sync.dma_start(out=s_d[:, ij, :], in_=s_sb[:])

        # encoder feats reload (transposed via DRAM): [ij, k, b]
        xenc = const.tile([FEAT, K, BL], BF16)
        nc.gpsimd.dma_start(out=xenc[:], in_=s_d.rearrange("k ij b -> ij k b"))

        # ================= shared pools for MLP phases =================
        psB = ctx.enter_context(tc.tile_pool(name="psB", bufs=2, space="PSUM"))
        psT = ctx.enter_context(tc.tile_pool(name="psT", bufs=2, space="PSUM"))
        psN = ctx.enter_context(tc.tile_pool(name="psN", bufs=4, space="PSUM"))
        work = ctx.enter_context(tc.tile_pool(name="work", bufs=2))
        chain = ctx.enter_context(tc.tile_pool(name="chain", bufs=3))
        smw = ctx.enter_context(tc.tile_pool(name="smw", bufs=3))

        state32 = const.tile([128, K, BL], F32)
        statebf = const.tile([128, K, BL], BF16)
        Ut = const.tile([128, K, 4, BL], BF16)
        Vt = const.tile([128, K, 4, BL], BF16)

        def ln_token_major(p2, mod):
            """psum [128tok, 512] -> t = LN(psum)*g + be (bf16, no relu)."""
            st6 = smw.tile([128, 6], F32, tag="ln_st")
            nc.vector.bn_stats(out=st6[:], in_=p2[:])
            mv = smw.tile([128, 2], F32, tag="ln_mv")
            nc.vector.bn_aggr(out=mv[:], in_=st6[:])
            sdv = smw.tile([128, 1], F32, tag="ln_sd")
            nc.scalar.activation(sdv[:], mv[:, 1:2], AF.Sqrt,
                                 bias=eps_t[:, 0:1])
            rs = smw.tile([128, 1], F32, tag="ln_rs")
            nc.vector.reciprocal(rs[:], sdv[:])
            mrs = smw.tile([128, 1], F32, tag="ln_mrs")
            nc.vector.tensor_scalar(out=mrs[:], in0=mv[:, 0:1],
                                    scalar1=rs[:, 0:1], scalar2=-1.0,
                                    op0=OP.mult, op1=OP.mult)
            xhat = chain.tile([128, HID], BF16, tag="ln_xhat")
            nc.scalar.activation(xhat[:], p2[:], AF.Identity,
                                 scale=rs[:, 0:1], bias=mrs[:, 0:1])
            xg = chain.tile([128, HID], BF16, tag="ln_xg")
            nc.vector.tensor_mul(xg[:], xhat[:], g_bc[mod][:])
            t = chain.tile([128, HID], BF16, tag="ln_t")
            nc.vector.tensor_add(t[:], xg[:], be_bc[mod][:])
            return t

        def l2_ln_t(src_fm, w2, mod, dstT):
            """token-major L2 + LN + transpose(+relu) back to feature-major.

            src_fm: [128, 4, HID/4-wide...] feature-major activation
                    (slices [.., kc, tc*128:+128] are the lhsT chunks)
            dstT:   [128, 4, HID] bf16 tile, gets LN-relu output transposed.
            """
            for tci in range(4):
                p2 = psB.tile([128, HID], F32, tag="pB")
                for kc in range(4):
                    nc.tensor.matmul(p2[:], src_fm[:, kc, bass.ts(tci, 128)],
                                     w2[:, kc, :],
                                     start=(kc == 0), stop=False)
                nc.tensor.matmul(p2[:], ones1[:, :], b2_row[mod][:, :],
                                 start=False, stop=True)
                t = ln_token_major(p2, mod)
                pt = psT.tile([128, 4, 128], BF16, tag="pT")
                for c in range(4):
                    nc.tensor.transpose(pt[:, c, :], t[:, bass.ts(c, 128)],
                                        ident[:])
                nc.scalar.activation(dstT[:, :, bass.ts(tci, 128)], pt[:],
                                     AF.Relu)

        # ================= encoder =================
        for g in range(K):
            h1e = work.tile([128, 4, HID], BF16, tag="h1e")
            for mc in range(4):
                p1 = psB.tile([128, HID], F32, tag="pB")
                nc.tensor.matmul(p1[:], W["enc_w1"][:, bass.ts(mc, 128)],
                                 xenc[:, g, :], start=True, stop=True)
                nc.scalar.activation(h1e[:, mc, :], p1[:], AF.Relu,
                                     bias=FC["enc_b1c"][:, mc:mc + 1])
            e2T = work.tile([128, 4, HID], BF16, tag="e2T")
            l2_ln_t(h1e, W["enc_w2"], "enc", e2T)
            p3 = psN.tile([128, BL], F32, tag="pN")
            for kc in range(4):
                nc.tensor.matmul(p3[:], W["enc_w3"][:, kc, :], e2T[:, kc, :],
                                 start=(kc == 0), stop=(kc == 3))
            nc.scalar.activation(state32[:, g, :], p3[:], AF.Identity,
                                 bias=FC["enc_b3c"][:, 0:1])
            nc.vector.tensor_scalar(out=statebf[:, g, :], in0=p3[:],
                                    scalar1=FC["enc_b3c"][:, 0:1],
                                    scalar2=None, op0=OP.add)

        # ================= edge U/V precompute (feature-major) =============
        for k in range(K):
            for mc in range(4):
                pu = psN.tile([128, BL], F32, tag="pN")
                nc.tensor.matmul(pu[:], W["edge_w1t"][:, bass.ts(mc, 128)],
                                 statebf[:, k, :], start=True, stop=True)
                nc.scalar.activation(Ut[:, k, mc, :], pu[:], AF.Identity,
                                     bias=FC["edge_b1c"][:, mc:mc + 1])
                pv = psN.tile([128, BL], F32, tag="pN")
                nc.tensor.matmul(pv[:], W["edge_w1b"][:, bass.ts(mc, 128)],
                                 statebf[:, k, :], start=True, stop=True)
                nc.scalar.activation(Vt[:, k, mc, :], pv[:], AF.Copy)

        # ============ edge MLP + W35 accumulation + node MLP ============
        for r in range(K):
            # node L1 psum accumulators (state + action first)
            pn = []
            for mc in range(4):
                p = psN.tile([128, BL], F32, tag="pN")
                nc.tensor.matmul(p[:], W["node_w1s"][:, bass.ts(mc, 128)],
                                 statebf[:, r, :], start=True, stop=False)
                nc.tensor.matmul(p[:], W["node_w1a"][:, bass.ts(mc, 128)],
                                 aohbf[:, r, :], start=False, stop=False)
                pn.append(p)
            for ei in range(4):
                c = COL[4 * r + ei]
                e1 = work.tile([128, 4, HID], BF16, tag="e1")
                for fcc in range(4):
                    nc.vector.tensor_add(e1[:, fcc, :], Ut[:, r, fcc, :],
                                         Vt[:, c, fcc, :])
                    nc.vector.tensor_scalar_max(e1[:, fcc, :], e1[:, fcc, :],
                                                0.0)
                e2T = work.tile([128, 4, HID], BF16, tag="e2T")
                l2_ln_t(e1, W["edge_w2"], "edge", e2T)
                for kc in range(4):
                    for mc in range(4):
                        nc.tensor.matmul(
                            pn[mc][:], W["w35"][:, kc, bass.ts(mc, 128)],
                            e2T[:, kc, :],
                            start=False, stop=(ei == 3 and kc == 3),
                        )
            # node MLP
            h1n = work.tile([128, 4, HID], BF16, tag="h1e")
            for mc in range(4):
                nc.scalar.activation(h1n[:, mc, :], pn[mc][:], AF.Relu,
                                     bias=FC["node_b1c"][:, mc:mc + 1])
            e2nT = work.tile([128, 4, HID], BF16, tag="e2T")
            l2_ln_t(h1n, W["node_w2"], "node", e2nT)
            pd = psN.tile([128, BL], F32, tag="pN")
            for kc in range(4):
                nc.tensor.matmul(pd[:], W["node_w3"][:, kc, :], e2nT[:, kc, :],
                                 start=(kc == 0), stop=(kc == 3))
            fo = chain.tile([128, BL], F32, tag="fo")
            nc.vector.affine_then_add(fo[:], in0=pd[:], in1=state32[:, r, :],
                                      scale=1.0, bias=FC["node_b3c"][:, 0:1])
            nc.sync.dma_start(out=out_d[r, :, :], in_=fo[:])


def _prep_inputs(obs, action, weights):
    """Host-side layout prep (permutations/reshapes/casts + linear folds)."""
    obs = np.ascontiguousarray(obs, dtype=np.float32)
    a = obs.reshape(NCORES, BL, 3, 5, 10, 5, 10)
    a = a.transpose(0, 2, 4, 6, 3, 5, 1)  # [core, c, di, dj, i, j, b]
    a = np.ascontiguousarray(a).reshape(NCORES, CIN, FEAT, BL)
    xT = np.zeros((NCORES, CINP, FEAT, BL), NPBF)
    xT[:, :CIN] = a.astype(NPBF)

    w = {k: np.asarray(v, dtype=np.float32) for k, v in weights.items()}

    w1c = w["conv1_w"].reshape(HID, CIN).T  # [300, 512]
    w1cp = np.zeros((CINP, HID), np.float32)
    w1cp[:CIN] = w1c
    w1c3 = np.ascontiguousarray(w1cp.reshape(3, 128, HID).transpose(1, 0, 2))

    def kc_tiles(m, kchunks):  # [K_in, N] -> [128, kchunks, N]
        return np.ascontiguousarray(
            m.reshape(kchunks, 128, m.shape[1]).transpose(1, 0, 2))

    def cols(v, n):  # [n*128] -> [128, n] per-partition chunk columns
        return np.ascontiguousarray(v.reshape(n, 128).T)

    w35 = w["edge_w3"] @ w["node_w1"][EMB + ADIM:]
    node_b1p = w["node_b1"] + 4.0 * (w["edge_b3"] @ w["node_w1"][EMB + ADIM:])

    bf = {
        "w1c": w1c3,
        "w2c": w["conv2_w"].reshape(K, HID).T.reshape(4, 128, K).transpose(1, 0, 2),
        "enc_w1": w["enc_w1"],
        "enc_w2": kc_tiles(w["enc_w2"], 4),
        "enc_w3": kc_tiles(w["enc_w3"], 4),
        "edge_w1t": w["edge_w1"][:EMB],
        "edge_w1b": w["edge_w1"][EMB:],
        "edge_w2": kc_tiles(w["edge_w2"], 4),
        "w35": kc_tiles(w35, 4),
        "node_w1s": w["node_w1"][:EMB],
        "node_w1a": w["node_w1"][EMB:EMB + ADIM],
        "node_w2": kc_tiles(w["node_w2"], 4),
        "node_w3": kc_tiles(w["node_w3"], 4),
        "enc_b2": w["enc_b2"], "edge_b2": w["edge_b2"], "node_b2": w["node_b2"],
        "enc_g": w["enc_lng"], "enc_be": w["enc_lnb"],
        "edge_g": w["edge_lng"], "edge_be": w["edge_lnb"],
        "node_g": w["node_lng"], "node_be": w["node_lnb"],
    }
    f32 = {
        "bng": cols(w["bn1_g"], 4), "bnb": cols(w["bn1_b"], 4),
        "b2c": w["conv2_b"].reshape(K, 1),
        "enc_b1c": cols(w["enc_b1"], 4),
        "edge_b1c": cols(w["edge_b1"], 4),
        "node_b1c": cols(node_b1p, 4),
        "enc_b3c": w["enc_b3"].reshape(EMB, 1),
        "node_b3c": w["node_b3"].reshape(EMB, 1),
    }

    act = np.asarray(action).astype(np.int64).reshape(NCORES, BL)
    k_idx = act // ADIM
    a_idx = act % ADIM
    aoh = np.zeros((NCORES, ADIM, K, BL), NPBF)
    core_i = np.repeat(np.arange(NCORES), BL)
    b_i = np.tile(np.arange(BL), NCORES)
    aoh[core_i, a_idx.ravel(), k_idx.ravel(), b_i] = 1.0

    shared = {k: np.ascontiguousarray(v.astype(NPBF)) for k, v in bf.items()}
    shared.update(
        {k: np.ascontiguousarray(v, dtype=np.float32) for k, v in f32.items()})

    in_maps = []
    for m in range(NCORES):
        d = dict(shared)
        d["xT"] = np.ascontiguousarray(xT[m])
        d["aoh"] = np.ascontiguousarray(aoh[m])
        in_maps.append(d)
    return in_maps


def kernel(**inputs):
    obs = inputs["obs"]
    action = inputs["action"]
    weights = {k: v for k, v in inputs.items()
               if k not in ("obs", "action")}
    in_maps = _prep_inputs(obs, action, weights)

    if "nc" not in _PROGRAM_CACHE:
        _PROGRAM_CACHE["nc"] = _build_program()
    nc = _PROGRAM_CACHE["nc"]

    res = run_bass_kernel_spmd(nc, in_maps, list(range(NCORES)))
    # out per core: [K, EMB, BL] -> [BL, K, EMB]
    out = np.concatenate(
        [np.transpose(res.results[m]["out"], (2, 0, 1)) for m in range(NCORES)],
        axis=0)
    return np.ascontiguousarray(out.astype(np.float32))


if __name__ == "__main__":
    print("built program OK" if _build_program() else "fail")


# revision 8
# speedup vs baseline: 2.1676x; 1.0324x over previous
"""ContrastiveSWM forward kernel for 8 trn2 NeuronCores (Bass/Tile), v2.

Data-parallel over batch (512 samples/core). Host does layout prep only
(im2col permutation, weight reshapes/casts, one-hot action, and the
linear fold W35 = edge_w3 @ node_w1g which removes the agg matmul).

Device pipeline (bf16 matmuls, f32 accumulation/LN math):
  conv1 (as matmul) -> stash h1 (bf16, SBUF) + batch stats
  -> tiny AllReduce -> BN affine+relu in place -> conv2 1x1 -> sigmoid
  -> encoder MLP -> edge MLP (feature-major in, token-major LN,
  transpose, W35 accumulate into node-L1 psum) -> node MLP -> state+delta.

Layout trick: L2 layers consume feature-major activations as the matmul
stationary operand, producing token-major outputs for cheap free-dim
LayerNorm; the post-LN transpose back to feature-major carries the relu
in its PSUM-evacuation copy.
"""
import sys

sys.path.insert(0, "/opt/trn_rl_repo")

import numpy as np
import ml_dtypes
import concourse.bass as bass
from concourse import bacc
import concourse.mybir as mybir
import concourse.tile as tile
from concourse.bass_utils import run_bass_kernel_spmd
from concourse.masks import make_identity

F32 = mybir.dt.float32
BF16 = mybir.dt.bfloat16
NPBF = ml_dtypes.bfloat16
AF = mybir.ActivationFunctionType
OP = mybir.AluOpType

NCORES = 8
B, K, EMB, HID, ADIM = 4096, 5, 128, 512, 4
BL = B // NCORES          # 512 samples per core
FEAT = 25                 # 5x5 spatial feature map
CIN = 300                 # 3*10*10 patch size
CINP = 384                # padded to 3*128
EPS = 1e-5

ROW = [0, 0, 0, 0, 1, 1, 1, 1, 2, 2, 2, 2, 3, 3, 3, 3, 4, 4, 4, 4]
COL = [1, 2, 3, 4, 0, 2, 3, 4, 0, 1, 3, 4, 0, 1, 2, 4, 0, 1, 2, 3]

_PROGRAM_CACHE = {}


def _build_program():
    nc = bacc.Bacc()

    # ---------------- dram I/O (per core) ----------------
    xT_d = nc.dram_tensor("xT", [CINP, FEAT, BL], BF16, kind="ExternalInput")
    aoh_d = nc.dram_tensor("aoh", [ADIM, K, BL], BF16, kind="ExternalInput")

    wb = {}   # bf16 weight tensors
    for name, shape in [
        ("w1c", [128, 3, HID]), ("w2c", [128, 4, K]),
        ("enc_w1", [FEAT, HID]), ("enc_w2", [128, 4, HID]),
        ("enc_w3", [128, 4, EMB]),
        ("edge_w1t", [128, HID]), ("edge_w1b", [128, HID]),
        ("edge_w2", [128, 4, HID]), ("w35", [128, 4, HID]),
        ("node_w1s", [128, HID]), ("node_w1a", [ADIM, HID]),
        ("node_w2", [128, 4, HID]), ("node_w3", [128, 4, EMB]),
    ]:
        wb[name] = nc.dram_tensor(name, shape, BF16, kind="ExternalInput")

    fc = {}   # f32 per-partition bias/param columns
    for name, shape in [
        ("bng", [128, 4]), ("bnb", [128, 4]), ("b2c", [K, 1]),
        ("enc_b1c", [128, 4]), ("edge_b1c", [128, 4]), ("node_b1c", [128, 4]),
        ("enc_b3c", [128, 1]), ("node_b3c", [128, 1]),
    ]:
        fc[name] = nc.dram_tensor(name, shape, F32, kind="ExternalInput")

    rows = {}  # bf16 rows [HID] for broadcast tiles / ones-matmul rhs
    for name in ["enc_b2", "edge_b2", "node_b2",
                 "enc_g", "enc_be", "edge_g", "edge_be", "node_g", "node_be"]:
        rows[name] = nc.dram_tensor(name, [HID], BF16, kind="ExternalInput")

    out_d = nc.dram_tensor("out", [K, EMB, BL], F32, kind="ExternalOutput")

    s_d = nc.dram_tensor("s_bounce", [K, FEAT, BL], BF16)  # sigmoid feats
    cc_in = nc.dram_tensor("cc_in", [128, 8], F32)
    cc_out = nc.dram_tensor("cc_out", [128, 8], F32, addr_space="Shared")

    with tile.TileContext(nc) as tc:
        _emit(nc, tc, xT_d, aoh_d, wb, fc, rows, out_d, s_d, cc_in, cc_out)
    nc.finalize()
    return nc


def _emit(nc, tc, xT_d, aoh_d, wb, fc, rows, out_d, s_d, cc_in, cc_out):
    from contextlib import ExitStack

    ctx = ExitStack()
    with ctx:
        const = ctx.enter_context(tc.tile_pool(name="const", bufs=1))
        sm = ctx.enter_context(tc.tile_pool(name="small", bufs=1))

        xT_v = xT_d.rearrange("(kc p) ij b -> p kc ij b", p=128)

        # conv1 weights first (needed by the very first matmuls)
        w1c = const.tile([128, 3, HID], BF16)
        nc.sync.dma_start(out=w1c[:], in_=wb["w1c"][:, :, :])

        # identity for PE transposes (bf16 via DVE cast)
        ident_f = const.tile([128, 128], F32)
        make_identity(nc, ident_f[:])
        ident = const.tile([128, 128], BF16)
        nc.vector.tensor_copy(out=ident[:], in_=ident_f[:])

        # PE warmup: dense dependency-free matmuls so the HAM un-throttles
        # while input DMAs stream; sink read keeps them from DCE.
        with tc.tile_pool(name="wps", bufs=1, space="PSUM") as wps:
            wp = wps.tile([128, 128], F32)
            for _ in range(40):
                nc.tensor.matmul(wp[:], ident[:], ident[:],
                                 start=True, stop=True)
            wsink = sm.tile([128, 1], F32)
            nc.scalar.activation(wsink[:], wp[:, 0:1], AF.Copy)

        ones1 = const.tile([1, 128], BF16)
        nc.vector.memset(ones1[:], 1.0)
        eps_t = const.tile([128, 1], F32)
        nc.vector.memset(eps_t[:], EPS)

        # remaining weights (gpsimd queue so x tiles stream on sync queue)
        W = {}
        for name, h in wb.items():
            if name == "w1c":
                W[name] = w1c
                continue
            t = const.tile(list(h.shape), BF16, tag=f"w_{name}")
            nc.gpsimd.dma_start(
                out=t[:], in_=h[tuple([slice(None)] * len(h.shape))])
            W[name] = t
        FC = {}
        for name, h in fc.items():
            t = const.tile(list(h.shape), F32, tag=f"fc_{name}")
            nc.gpsimd.dma_start(out=t[:], in_=h[:, :])
            FC[name] = t

        def bc_row(src_h, _tag=[0]):
            """[HID] bf16 dram row -> [128, HID] broadcast tile."""
            _tag[0] += 1
            dst = const.tile([128, HID], BF16, tag=f"bcrow{_tag[0]}")
            ap = src_h.ap()
            bcast = bass.AP(tensor=ap.tensor, offset=ap.offset,
                            ap=[[0, 128]] + ap.ap)
            nc.gpsimd.dma_start(out=dst[:], in_=bcast)
            return dst

        def one_row(src_h, _tag=[0]):
            """[HID] bf16 dram row -> [1, HID] tile (ones-matmul rhs)."""
            _tag[0] += 1
            dst = const.tile([1, HID], BF16, tag=f"onerow{_tag[0]}")
            nc.gpsimd.dma_start(
                out=dst[:], in_=src_h.ap().rearrange("(o f) -> o f", o=1))
            return dst

        g_bc = {m: bc_row(rows[f"{m}_g"]) for m in ("enc", "edge", "node")}
        be_bc = {m: bc_row(rows[f"{m}_be"]) for m in ("enc", "edge", "node")}
        b2_row = {m: one_row(rows[f"{m}_b2"]) for m in ("enc", "edge", "node")}

        aohbf = const.tile([ADIM, K, BL], BF16)
        nc.gpsimd.dma_start(out=aohbf[:], in_=aoh_d[:, :, :])

        # ================= conv phase =================
        stats_all = sm.tile([128, 4, FEAT, 6], F32)
        P = sm.tile([128, 4, 2], F32)

        with (
            tc.tile_pool(name="xtile", bufs=3) as xpool,
            tc.tile_pool(name="cps", bufs=4, space="PSUM") as cps,
        ):
            # conv1 (no bias) -> stash + per-channel batch stats
            for ij in range(FEAT):
                xt = xpool.tile([128, 3, BL], BF16)
                nc.sync.dma_start(out=xt[:], in_=xT_v[:, :, ij, :])
                for cc in range(4):
                    hp = cps.tile([128, BL], F32)
                    for kc in range(3):
                        nc.tensor.matmul(
                            hp[:], w1c[:, kc, bass.ts(cc, 128)], xt[:, kc, :],
                            start=(kc == 0), stop=(kc == 2),
                        )
                    nc.vector.bn_stats(out=stats_all[:, cc, ij, :], in_=hp[:])

        # aggregate per-channel mean/var; q = var + mean^2
        for cc in range(4):
            nc.vector.bn_aggr(out=P[:, cc, :], in_=stats_all[:, cc, :, :])
        msq = sm.tile([128, 4], F32)
        nc.vector.tensor_mul(msq[:], P[:, :, 0], P[:, :, 0])
        nc.vector.tensor_add(P[:, :, 1], P[:, :, 1], msq[:])

        nc.sync.dma_start(out=cc_in[:, :], in_=P[:].rearrange("p a b -> p (a b)"))
        nc.gpsimd.collective_compute(
            "AllReduce", OP.add,
            replica_groups=[list(range(NCORES))],
            ins=[cc_in[:, :]], outs=[cc_out[:, :]],
        )
        G = sm.tile([128, 4, 2], F32)
        nc.sync.dma_start(out=G[:].rearrange("p a b -> p (a b)"), in_=cc_out[:, :])

        mg = sm.tile([128, 4], F32)
        qg = sm.tile([128, 4], F32)
        nc.vector.tensor_scalar(out=mg[:], in0=G[:, :, 0], scalar1=1.0 / NCORES,
                                scalar2=None, op0=OP.mult)
        nc.vector.tensor_scalar(out=qg[:], in0=G[:, :, 1], scalar1=1.0 / NCORES,
                                scalar2=None, op0=OP.mult)
        varg = sm.tile([128, 4], F32)
        nc.vector.tensor_mul(varg[:], mg[:], mg[:])
        nc.vector.tensor_sub(varg[:], qg[:], varg[:])
        sd = sm.tile([128, 4], F32)
        nc.scalar.activation(sd[:], varg[:], AF.Sqrt, bias=eps_t[:, 0:1])
        rstd = sm.tile([128, 4], F32)
        nc.vector.reciprocal(rstd[:], sd[:])
        scale = sm.tile([128, 4], F32)
        nc.vector.tensor_mul(scale[:], FC["bng"][:], rstd[:])
        shift = sm.tile([128, 4], F32)
        nc.vector.tensor_mul(shift[:], mg[:], scale[:])
        nc.vector.tensor_sub(shift[:], FC["bnb"][:], shift[:])

        # pass 2: conv1 recompute -> BN+relu -> conv2 -> sigmoid.
        # First RAW_IJ tiles evacuate raw (no BN dependency) so the PE keeps
        # streaming through the ~30us AllReduce; their BN runs on DVE after.
        RAW_IJ = 10
        with (
            tc.tile_pool(name="xtile2", bufs=4) as xpool,
            tc.tile_pool(name="cps2", bufs=5, space="PSUM") as cps,
            tc.tile_pool(name="h1rp", bufs=RAW_IJ) as h1rp,
            tc.tile_pool(name="hbnp", bufs=3) as hbnp,
            tc.tile_pool(name="sps", bufs=2, space="PSUM") as sps,
            tc.tile_pool(name="ssb", bufs=3) as ssb,
        ):
            def conv2_sig(ij, hbn):
                sp = sps.tile([K, BL], F32)
                for cc in range(4):
                    nc.tensor.matmul(
                        sp[:], W["w2c"][:, cc, :], hbn[:, cc, :],
                        start=(cc == 0), stop=(cc == 3),
                    )
                s_sb = ssb.tile([K, BL], BF16)
                nc.scalar.activation(s_sb[:], sp[:], AF.Sigmoid,
                                     bias=FC["b2c"][:, 0:1])
                nc.sync.dma_start(out=s_d[:, ij, :], in_=s_sb[:])

            raw_tiles = []
            for ij in range(FEAT):
                xt = xpool.tile([128, 3, BL], BF16)
                nc.sync.dma_start(out=xt[:], in_=xT_v[:, :, ij, :])
                raw = ij < RAW_IJ
                if raw:
                    dst = h1rp.tile([128, 4, BL], BF16, tag="h1r")
                else:
                    dst = hbnp.tile([128, 4, BL], BF16, tag="hbn")
                for cc in range(4):
                    hp = cps.tile([128, BL], F32)
                    for kc in range(3):
                        nc.tensor.matmul(
                            hp[:], w1c[:, kc, bass.ts(cc, 128)], xt[:, kc, :],
                            start=(kc == 0), stop=(kc == 2),
                        )
                    if raw:
                        nc.scalar.activation(dst[:, cc, :], hp[:], AF.Copy)
                    else:
                        nc.scalar.activation(dst[:, cc, :], hp[:], AF.Relu,
                                             scale=scale[:, cc:cc + 1],
                                             bias=shift[:, cc:cc + 1])
                if raw:
                    raw_tiles.append(dst)
                else:
                    conv2_sig(ij, dst)
            for ij in range(RAW_IJ):
                h1r = raw_tiles[ij]
                hbn = hbnp.tile([128, 4, BL], BF16, tag="hbn")
                for cc in range(4):
                    nc.vector.tensor_scalar(
                        out=hbn[:, cc, :], in0=h1r[:, cc, :],
                        scalar1=scale[:, cc:cc + 1],
                        scalar2=shift[:, cc:cc + 1],
                        op0=OP.mult, op1=OP.add)
                    nc.vector.tensor_scalar_max(hbn[:, cc, :], hbn[:, cc, :],
                                                0.0)
                conv2_sig(ij, hbn)

        # encoder feats reload: s_d[g] is already [ij, b] for object g
        xenc = const.tile([FEAT, K, BL], BF16)
        for g in range(K):
            nc.gpsimd.dma_start(out=xenc[:, g, :], in_=s_d[g, :, :])

        # ================= shared pools for MLP phases =================
        psB = ctx.enter_context(tc.tile_pool(name="psB", bufs=2, space="PSUM"))
        psT = ctx.enter_context(tc.tile_pool(name="psT", bufs=2, space="PSUM"))
        psN = ctx.enter_context(tc.tile_pool(name="psN", bufs=4, space="PSUM"))
        work = ctx.enter_context(tc.tile_pool(name="work", bufs=3))
        chain = ctx.enter_context(tc.tile_pool(name="chain", bufs=3))
        smw = ctx.enter_context(tc.tile_pool(name="smw", bufs=3))

        state32 = const.tile([128, K, BL], F32)
        statebf = const.tile([128, K, BL], BF16)
        Ut = const.tile([128, K, 4, BL], BF16)
        Vt = const.tile([128, K, 4, BL], BF16)

        def ln_token_major(p2, mod):
            """psum [128tok, 512] -> t = LN(psum)*g + be (bf16, no relu)."""
            st6 = smw.tile([128, 6], F32, tag="ln_st")
            nc.vector.bn_stats(out=st6[:], in_=p2[:])
            mv = smw.tile([128, 2], F32, tag="ln_mv")
            nc.vector.bn_aggr(out=mv[:], in_=st6[:])
            sdv = smw.tile([128, 1], F32, tag="ln_sd")
            nc.scalar.activation(sdv[:], mv[:, 1:2], AF.Sqrt,
                                 bias=eps_t[:, 0:1])
            rs = smw.tile([128, 1], F32, tag="ln_rs")
            nc.vector.reciprocal(rs[:], sdv[:])
            mrs = smw.tile([128, 1], F32, tag="ln_mrs")
            nc.vector.tensor_scalar(out=mrs[:], in0=mv[:, 0:1],
                                    scalar1=rs[:, 0:1], scalar2=-1.0,
                                    op0=OP.mult, op1=OP.mult)
            xhat = chain.tile([128, HID], BF16, tag="ln_xhat")
            nc.scalar.activation(xhat[:], p2[:], AF.Identity,
                                 scale=rs[:, 0:1], bias=mrs[:, 0:1])
            xg = chain.tile([128, HID], BF16, tag="ln_xg")
            nc.vector.tensor_mul(xg[:], xhat[:], g_bc[mod][:])
            t = chain.tile([128, HID], BF16, tag="ln_t")
            nc.vector.tensor_add(t[:], xg[:], be_bc[mod][:])
            return t

        def l2_ln_t(src_fm, w2, mod, dstT):
            """token-major L2 + LN + transpose(+relu) back to feature-major.

            src_fm: [128, 4, HID/4-wide...] feature-major activation
                    (slices [.., kc, tc*128:+128] are the lhsT chunks)
            dstT:   [128, 4, HID] bf16 tile, gets LN-relu output transposed.
            """
            for tci in range(4):
                p2 = psB.tile([128, HID], F32, tag="pB")
                for kc in range(4):
                    nc.tensor.matmul(p2[:], src_fm[:, kc, bass.ts(tci, 128)],
                                     w2[:, kc, :],
                                     start=(kc == 0), stop=False)
                nc.tensor.matmul(p2[:], ones1[:, :], b2_row[mod][:, :],
                                 start=False, stop=True)
                t = ln_token_major(p2, mod)
                pt = psT.tile([128, 4, 128], BF16, tag="pT")
                for c in range(4):
                    nc.tensor.transpose(pt[:, c, :], t[:, bass.ts(c, 128)],
                                        ident[:])
                nc.scalar.activation(dstT[:, :, bass.ts(tci, 128)], pt[:],
                                     AF.Relu)

        # ================= encoder =================
        for g in range(K):
            h1e = work.tile([128, 4, HID], BF16, tag="h1e")
            for mc in range(4):
                p1 = psB.tile([128, HID], F32, tag="pB")
                nc.tensor.matmul(p1[:], W["enc_w1"][:, bass.ts(mc, 128)],
                                 xenc[:, g, :], start=True, stop=True)
                nc.scalar.activation(h1e[:, mc, :], p1[:], AF.Relu,
                                     bias=FC["enc_b1c"][:, mc:mc + 1])
            e2T = work.tile([128, 4, HID], BF16, tag="e2T")
            l2_ln_t(h1e, W["enc_w2"], "enc", e2T)
            p3 = psN.tile([128, BL], F32, tag="pN")
            for kc in range(4):
                nc.tensor.matmul(p3[:], W["enc_w3"][:, kc, :], e2T[:, kc, :],
                                 start=(kc == 0), stop=(kc == 3))
            nc.scalar.activation(state32[:, g, :], p3[:], AF.Identity,
                                 bias=FC["enc_b3c"][:, 0:1])
            nc.vector.tensor_scalar(out=statebf[:, g, :], in0=p3[:],
                                    scalar1=FC["enc_b3c"][:, 0:1],
                                    scalar2=None, op0=OP.add)

        # ================= edge U/V precompute (feature-major) =============
        for k in range(K):
            for mc in range(4):
                pu = psN.tile([128, BL], F32, tag="pN")
                nc.tensor.matmul(pu[:], W["edge_w1t"][:, bass.ts(mc, 128)],
                                 statebf[:, k, :], start=True, stop=True)
                nc.scalar.activation(Ut[:, k, mc, :], pu[:], AF.Identity,
                                     bias=FC["edge_b1c"][:, mc:mc + 1])
                pv = psN.tile([128, BL], F32, tag="pN")
                nc.tensor.matmul(pv[:], W["edge_w1b"][:, bass.ts(mc, 128)],
                                 statebf[:, k, :], start=True, stop=True)
                nc.scalar.activation(Vt[:, k, mc, :], pv[:], AF.Copy)

        # ============ edge MLP + W35 accumulation + node MLP ============
        for r in range(K):
            # node L1 psum accumulators (state + action first)
            pn = []
            for mc in range(4):
                p = psN.tile([128, BL], F32, tag="pN")
                nc.tensor.matmul(p[:], W["node_w1s"][:, bass.ts(mc, 128)],
                                 statebf[:, r, :], start=True, stop=False)
                nc.tensor.matmul(p[:], W["node_w1a"][:, bass.ts(mc, 128)],
                                 aohbf[:, r, :], start=False, stop=False)
                pn.append(p)
            for ei in range(4):
                c = COL[4 * r + ei]
                e1 = work.tile([128, 4, HID], BF16, tag="e1")
                for fcc in range(4):
                    nc.vector.tensor_add(e1[:, fcc, :], Ut[:, r, fcc, :],
                                         Vt[:, c, fcc, :])
                    nc.vector.tensor_scalar_max(e1[:, fcc, :], e1[:, fcc, :],
                                                0.0)
                e2T = work.tile([128, 4, HID], BF16, tag="e2T")
                l2_ln_t(e1, W["edge_w2"], "edge", e2T)
                for kc in range(4):
                    for mc in range(4):
                        nc.tensor.matmul(
                            pn[mc][:], W["w35"][:, kc, bass.ts(mc, 128)],
                            e2T[:, kc, :],
                            start=False, stop=(ei == 3 and kc == 3),
                        )
            # node MLP
            h1n = work.tile([128, 4, HID], BF16, tag="h1e")
            for mc in range(4):
                nc.scalar.activation(h1n[:, mc, :], pn[mc][:], AF.Relu,
                                     bias=FC["node_b1c"][:, mc:mc + 1])
            e2nT = work.tile([128, 4, HID], BF16, tag="e2T")
            l2_ln_t(h1n, W["node_w2"], "node", e2nT)
            pd = psB.tile([128, BL], F32, tag="pB")
            for kc in range(4):
                nc.tensor.matmul(pd[:], W["node_w3"][:, kc, :], e2nT[:, kc, :],
                                 start=(kc == 0), stop=(kc == 3))
            fo = chain.tile([128, BL], F32, tag="fo")
            nc.vector.affine_then_add(fo[:], in0=pd[:], in1=state32[:, r, :],
                                      scale=1.0, bias=FC["node_b3c"][:, 0:1])
            nc.sync.dma_start(out=out_d[r, :, :], in_=fo[:])


def _prep_inputs(obs, action, weights):
    """Host-side layout prep (permutations/reshapes/casts + linear folds)."""
    obs = np.ascontiguousarray(obs, dtype=np.float32)
    a = obs.reshape(NCORES, BL, 3, 5, 10, 5, 10)
    a = a.transpose(0, 2, 4, 6, 3, 5, 1)  # [core, c, di, dj, i, j, b]
    a = np.ascontiguousarray(a).reshape(NCORES, CIN, FEAT, BL)
    xT = np.zeros((NCORES, CINP, FEAT, BL), NPBF)
    xT[:, :CIN] = a.astype(NPBF)

    w = {k: np.asarray(v, dtype=np.float32) for k, v in weights.items()}

    w1c = w["conv1_w"].reshape(HID, CIN).T  # [300, 512]
    w1cp = np.zeros((CINP, HID), np.float32)
    w1cp[:CIN] = w1c
    w1c3 = np.ascontiguousarray(w1cp.reshape(3, 128, HID).transpose(1, 0, 2))

    def kc_tiles(m, kchunks):  # [K_in, N] -> [128, kchunks, N]
        return np.ascontiguousarray(
            m.reshape(kchunks, 128, m.shape[1]).transpose(1, 0, 2))

    def cols(v, n):  # [n*128] -> [128, n] per-partition chunk columns
        return np.ascontiguousarray(v.reshape(n, 128).T)

    w35 = w["edge_w3"] @ w["node_w1"][EMB + ADIM:]
    node_b1p = w["node_b1"] + 4.0 * (w["edge_b3"] @ w["node_w1"][EMB + ADIM:])

    bf = {
        "w1c": w1c3,
        "w2c": w["conv2_w"].reshape(K, HID).T.reshape(4, 128, K).transpose(1, 0, 2),
        "enc_w1": w["enc_w1"],
        "enc_w2": kc_tiles(w["enc_w2"], 4),
        "enc_w3": kc_tiles(w["enc_w3"], 4),
        "edge_w1t": w["edge_w1"][:EMB],
        "edge_w1b": w["edge_w1"][EMB:],
        "edge_w2": kc_tiles(w["edge_w2"], 4),
        "w35": kc_tiles(w35, 4),
        "node_w1s": w["node_w1"][:EMB],
        "node_w1a": w["node_w1"][EMB:EMB + ADIM],
        "node_w2": kc_tiles(w["node_w2"], 4),
        "node_w3": kc_tiles(w["node_w3"], 4),
        "enc_b2": w["enc_b2"], "edge_b2": w["edge_b2"], "node_b2": w["node_b2"],
        "enc_g": w["enc_lng"], "enc_be": w["enc_lnb"],
        "edge_g": w["edge_lng"], "edge_be": w["edge_lnb"],
        "node_g": w["node_lng"], "node_be": w["node_lnb"],
    }
    f32 = {
        "bng": cols(w["bn1_g"], 4), "bnb": cols(w["bn1_b"], 4),
        "b2c": w["conv2_b"].reshape(K, 1),
        "enc_b1c": cols(w["enc_b1"], 4),
        "edge_b1c": cols(w["edge_b1"], 4),
        "node_b1c": cols(node_b1p, 4),
        "enc_b3c": w["enc_b3"].reshape(EMB, 1),
        "node_b3c": w["node_b3"].reshape(EMB, 1),
    }

    act = np.asarray(action).astype(np.int64).reshape(NCORES, BL)
    k_idx = act // ADIM
    a_idx = act % ADIM
    aoh = np.zeros((NCORES, ADIM, K, BL), NPBF)
    core_i = np.repeat(np.arange(NCORES), BL)
    b_i = np.tile(np.arange(BL), NCORES)
    aoh[core_i, a_idx.ravel(), k_idx.ravel(), b_i] = 1.0

    shared = {k: np.ascontiguousarray(v.astype(NPBF)) for k, v in bf.items()}
    shared.update(
        {k: np.ascontiguousarray(v, dtype=np.float32) for k, v in f32.items()})

    in_maps = []
    for m in range(NCORES):
        d = dict(shared)
        d["xT"] = np.ascontiguousarray(xT[m])
        d["aoh"] = np.ascontiguousarray(aoh[m])
        in_maps.append(d)
    return in_maps


def kernel(**inputs):
    obs = inputs["obs"]
    action = inputs["action"]
    weights = {k: v for k, v in inputs.items()
               if k not in ("obs", "action")}
    in_maps = _prep_inputs(obs, action, weights)

    if "nc" not in _PROGRAM_CACHE:
        _PROGRAM_CACHE["nc"] = _build_program()
    nc = _PROGRAM_CACHE["nc"]

    res = run_bass_kernel_spmd(nc, in_maps, list(range(NCORES)))
    # out per core: [K, EMB, BL] -> [BL, K, EMB]
    out = np.concatenate(
        [np.transpose(res.results[m]["out"], (2, 0, 1)) for m in range(NCORES)],
        axis=0)
    return np.ascontiguousarray(out.astype(np.float32))


if __name__ == "__main__":
    print("built program OK" if _build_program() else "fail")
